# revision 2
# baseline (speedup 1.0000x reference)
"""Trainium2 Bass kernel for nn_MessagePassing_300647711374 — fp8 DoubleRow.

Sharding: 8 cores = 2 batches x 4 D-chunks of 32 planes (+3 halo planes,
zeros baked by the host outside the global domain).

Device layout per core: partitions = H(128); columns = (channel, w_ext, d)
with w_ext in [0,130) (zero guard w-planes at 0/129, data w = w_ext-1) and
d in [0,38). One fp8 buffer holds hi and lo blocks of all 6 input
channels, where x = fp8(x) + fp8(x - fp8(x)) (hi/lo split keeps x at
~bf16 accuracy through fp8 matmuls). A 3x3x3 conv = 9 (dz,dw) taps; each
tap is a 128x128 H-band matmul over contiguous (w,d) column windows, run
for both hi and lo blocks. Tap pairs pack 2 matmuls per instruction with
fp8e4m3 DoubleRow perf mode (0.5 cycles/output column): single convs are
exactly 9 DR (4 hi pairs, 4 lo pairs, and (hi_t8, lo_t8) chained via a
duplicated band slot); two-conv entries pair across convs (18 DR).
dz/dw window shifts contaminate only halo outputs (d<3, d>=35), never
stored.

Chain intermediates (f5n,f7n,f8n,f10n) are recomputed on device in fp8
hi/lo (DVE: t = psum + hi + lo; hi' = fp8(t); lo' = fp8(t - hi')); all 10
output channels return pure conv *deltas* in bf16; the host adds them to
the f32 input and recomputes the global-D-edge planes of chained channels
exactly (reference zero-pads intermediates at the domain edge, which a
halo'd chain cannot reproduce).
"""
import numpy as np
import ml_dtypes

import concourse.bass as bass
import concourse.tile as tile
from concourse import bacc, mybir
from concourse.ap import AP
from concourse.bass2jax import _bass_exec_p, install_neuronx_cc_hook, partition_id_tensor

P = 128
D_CHUNK = 32
HALO = 3
EXT = D_CHUNK + 2 * HALO       # 38
WE = 130                       # w_ext: zero guard w-planes at 0 and 129
CW = WE * EXT                  # 4940 cols per channel block
G = 1                          # leading guard col (tap offset can be -1)
LO = 6 * CW                    # lo block offset
XW = G + 12 * CW + 1           # hi block + lo block + guards
I0, I1 = HALO, HALO + D_CHUNK  # kept d range [3, 35)
N_CORES = 8
NW = 6                         # w-planes per psum group (N = 228, 2N <= 512)

# taps sorted by column offset dw*EXT+dz so DR pair strides are positive
TAPS = [(dz, dw) for dw in (-1, 0, 1) for dz in (-1, 0, 1)]

CH_IN = [4, 5, 6, 7, 10, 8]    # channel buffer order (k) -> feature channel
CH_OUT = [0, 1, 2, 3, 5, 7, 8, 10, 12, 13]
DMAP = {c: i for i, c in enumerate(CH_OUT)}

# entries: convs = [(weight_idx, src_k), ...] (2 convs = 18 DR pairs),
# delta = output channel, inter = src_k buffer to add psum into (in place)
ENTRIES = [
    dict(convs=[(1, 0), (0, 1)], delta=0),    # conv02(x4)+conv01(x5) -> f0
    dict(convs=[(2, 0)], delta=5, inter=1),   # conv50(x4) -> f5n into x5
    dict(convs=[(5, 2), (4, 3)], delta=1),    # conv11(x6)+conv10(x7) -> f1
    dict(convs=[(6, 2)], delta=7, inter=3),   # conv70(x6) -> f7n into x7
    dict(convs=[(8, 4)], delta=8, inter=5),   # conv80(x10) -> f8n into x8
    dict(convs=[(3, 1)], delta=2),            # conv20(f5n)   (covers E5 DVE)
    dict(convs=[(9, 5)], delta=10, inter=4),  # conv100(f8n) -> f10n into x10
    dict(convs=[(7, 3)], delta=3),            # conv30(f7n)
    dict(convs=[(10, 5)], delta=12),          # conv120(f8n)
    dict(convs=[(11, 4)], delta=13),          # conv130(f10n)
]

# band slots: pair entries 18 (A taps 0-8, B taps 0-8); singles 10
# (taps 0-8 plus tap 8 duplicated at slot 9 for the (hi_t8, lo_t8) DR)
_SLOT_BASE = []
_ns = 0
for _e in ENTRIES:
    _SLOT_BASE.append(_ns)
    _ns += 18 if len(_e["convs"]) == 2 else 10
N_SLOTS = _ns  # 116


def _tap_off(dz, dw):
    return dw * EXT + dz


def groups_w():
    out, w0 = [], 0
    while w0 < P:
        nw = min(NW, P - w0)
        out.append((w0, nw))
        w0 += nw
    return out


# ---------------------------------------------------------------- device ---
def build_nc(reps=1, ablate=()):
    """ablate: subset of {'in_dma','mm','adds','out'} to skip (timing probes)."""
    from contextlib import ExitStack
    f32 = mybir.dt.float32
    f8 = mybir.dt.float8e4
    bf16 = mybir.dt.bfloat16
    DR = mybir.MatmulPerfMode.DoubleRow

    nc = bacc.Bacc("TRN2", target_bir_lowering=False, debug=False,
                   num_devices=N_CORES)
    xin = nc.dram_tensor("xin", [P, XW], f8, kind="ExternalInput").ap()
    bands = nc.dram_tensor("bands", [P, N_SLOTS * P], f8,
                           kind="ExternalInput").ap()
    deltas = nc.dram_tensor("deltas", [10, P, P * D_CHUNK], bf16,
                            kind="ExternalOutput").ap()

    with tile.TileContext(nc) as tc:
        with ExitStack() as ctx:
            sb = ctx.enter_context(tc.tile_pool(name="sb", bufs=1))
            psum = ctx.enter_context(tc.tile_pool(name="ps", bufs=8, space="PSUM"))

            def body(_it):
                chan = sb.tile([P, XW], f8, tag="chan", bufs=2, name="chan")
                ball = sb.tile([P, N_SLOTS * P], f8, tag="ball", bufs=2,
                               name="ball")
                pdim = list(chan.ap)[0]
                bdim = list(ball.ap)[0]

                nc.sync.dma_start(ball, bands[:, :])
                if 'in_dma' not in ablate:
                    for k0 in (0, 2, 4):
                        a = G + k0 * CW
                        lo0 = a - G if k0 == 0 else a
                        nc.sync.dma_start(chan[:, lo0:a + 2 * CW],
                                          xin[:, lo0:a + 2 * CW])
                        al = a + LO
                        hi1 = al + 2 * CW + (1 if k0 == 4 else 0)
                        nc.sync.dma_start(chan[:, al:hi1], xin[:, al:hi1])

                def mk_rhs(off0, dr, N):
                    return AP(chan.tensor, chan.offset + off0,
                              [pdim, [dr, 2], [1, N]])

                def mk_lhs(s0, dr_slots):
                    return AP(ball.tensor, ball.offset + s0 * P,
                              [bdim, [dr_slots * P, 2], [1, P]])

                def run_entry(ei, ent, w0, nw):
                    N = nw * EXT
                    acc = psum.tile([P, NW * EXT], f32, tag="ps",
                                    name="acc")[:, 0:N]
                    sbase = _SLOT_BASE[ei]
                    convs = ent["convs"]
                    if 'mm' not in ablate:
                        def boff(k_a, dz, dw):
                            return G + k_a * CW + (1 + w0 + dw) * EXT + dz
                        if len(convs) == 2:
                            (w_a, k_a), (w_b, k_b) = convs
                            drr = (k_b - k_a) * CW
                            mm = 0
                            for blk in (0, LO):      # hi block, lo block
                                for i, (dz, dw) in enumerate(TAPS):
                                    nc.tensor.matmul(
                                        acc, mk_lhs(sbase + i, 9),
                                        mk_rhs(boff(k_a, dz, dw) + blk, drr, N),
                                        start=(mm == 0), stop=(mm == 17),
                                        perf_mode=DR)
                                    mm += 1
                        else:
                            (w_a, k_a), = convs
                            # 9 DR: 4 hi pairs, 4 lo pairs, (hi_t8, lo_t8)
                            plan = []
                            for j in range(4):
                                t0, t1 = TAPS[2 * j], TAPS[2 * j + 1]
                                d = _tap_off(*t1) - _tap_off(*t0)
                                plan.append((sbase + 2 * j, 1,
                                             boff(k_a, *t0), d))
                                plan.append((sbase + 2 * j, 1,
                                             boff(k_a, *t0) + LO, d))
                            plan.append((sbase + 8, 1,
                                         boff(k_a, *TAPS[8]), LO))
                            for mm, (s0, ds, o0, drr) in enumerate(plan):
                                nc.tensor.matmul(
                                    acc, mk_lhs(s0, ds), mk_rhs(o0, drr, N),
                                    start=(mm == 0), stop=(mm == len(plan) - 1),
                                    perf_mode=DR)
                    if "inter" in ent and 'adds' not in ablate:
                        kd = ent["inter"]
                        base = G + kd * CW + (1 + w0) * EXT
                        hi = chan[:, base:base + N]
                        lo = chan[:, base + LO:base + LO + N]
                        t = sb.tile([P, NW * EXT], f32, tag="scr",
                                    bufs=4, name="t")[:, 0:N]
                        nc.vector.tensor_add(t, acc, hi)
                        nc.vector.tensor_add(t, t, lo)
                        nc.vector.tensor_copy(hi, t)
                        nc.vector.tensor_sub(lo, t, hi)
                    return acc

                for ei, ent in enumerate(ENTRIES):
                    kd = DMAP[ent["delta"]]
                    st = sb.tile([P, P * D_CHUNK], bf16, tag="stage", bufs=3,
                                 name="st")
                    for (w0, nw) in groups_w():
                        acc = run_entry(ei, ent, w0, nw)
                        if 'out' in ablate:
                            continue
                        src3 = acc.rearrange("p (w d) -> p w d",
                                             d=EXT)[:, :, I0:I1]
                        dst3 = st[:, w0 * D_CHUNK:(w0 + nw) * D_CHUNK
                                  ].rearrange("p (w d) -> p w d", d=D_CHUNK)
                        if ei % 2 == 0:
                            nc.scalar.mul(dst3, src3, 1.0)
                        else:
                            nc.vector.tensor_copy(dst3, src3)
                    if 'out' not in ablate:
                        nc.sync.dma_start(deltas[kd], st)

            if reps > 1:
                with tc.For_i(0, reps, 1) as it:
                    body(it)
            else:
                body(0)
    nc.compile()
    return nc


# ------------------------------------------------------------------ host ---
def build_bands(weights):
    weights = np.asarray(weights, dtype=np.float32)
    bands = np.zeros((N_SLOTS, P, P), dtype=np.float32)
    eyes = {d: np.eye(P, k=-d, dtype=np.float32) for d in (-1, 0, 1)}

    def band(wi, dz, dw):
        return sum(weights[wi, dz + 1, dy + 1, dw + 1] * eyes[dy]
                   for dy in (-1, 0, 1))

    for ei, ent in enumerate(ENTRIES):
        sbase = _SLOT_BASE[ei]
        for c, (wi, _k) in enumerate(ent["convs"]):
            for i, (dz, dw) in enumerate(TAPS):
                bands[sbase + 9 * c + i] = band(wi, dz, dw)
        if len(ent["convs"]) == 1:
            wi = ent["convs"][0][0]
            bands[sbase + 9] = band(wi, *TAPS[8])
    return np.ascontiguousarray(
        bands.transpose(1, 0, 2).reshape(P, N_SLOTS * P)
    ).astype(ml_dtypes.float8_e4m3)


def make_shards(feature):
    feature = np.asarray(feature, dtype=np.float32)
    shards = []
    for c in range(N_CORES):
        b, q = divmod(c, 4)
        d0 = q * D_CHUNK - HALO
        buf = np.zeros((P, 6, WE, EXT), dtype=np.float32)
        lo_, hi_ = max(d0, 0), min(d0 + EXT, P)
        for k, ch in enumerate(CH_IN):
            # [D',H,W] -> [H, W, D']
            buf[:, k, 1:129, lo_ - d0:hi_ - d0] = \
                feature[b, ch, lo_:hi_].transpose(1, 2, 0)
        flat = buf.reshape(P, 6 * CW)
        xhi = flat.astype(ml_dtypes.float8_e4m3)
        xlo = (flat - xhi.astype(np.float32)).astype(ml_dtypes.float8_e4m3)
        xin = np.zeros((P, XW), dtype=ml_dtypes.float8_e4m3)
        xin[:, G:G + LO] = xhi
        xin[:, G + LO:G + 2 * LO] = xlo
        shards.append(xin)
    return shards


def _shift2(pl, dy, dw):
    out = np.zeros_like(pl)
    out[max(-dy, 0):P + min(-dy, 0), max(-dw, 0):P + min(-dw, 0)] = \
        pl[max(dy, 0):P + min(dy, 0), max(dw, 0):P + min(dw, 0)]
    return out


def _cp(getter, wk, d):
    acc = np.zeros((P, P), np.float32)
    for dz in (-1, 0, 1):
        p = d + dz
        if not 0 <= p < P:
            continue
        pl = getter(p)
        for dy in (-1, 0, 1):
            for dw in (-1, 0, 1):
                acc += wk[dz + 1, dy + 1, dw + 1] * _shift2(pl, dy, dw)
    return acc


def fix_boundaries(out, feature, weights):
    """Recompute global-D-edge planes of chained channels with exact
    reference semantics (intermediates zeroed outside the domain)."""
    DFIX = [0, 1, 126, 127]
    w50, w20, w70, w30 = weights[2], weights[3], weights[6], weights[7]
    w80, w100, w120, w130 = weights[8], weights[9], weights[10], weights[11]
    for b in range(feature.shape[0]):
        f = feature[b]

        def cache(fn):
            c = {}
            def g(p):
                if p not in c:
                    c[p] = fn(p)
                return c[p]
            return g

        f5n = cache(lambda p: f[5][p] + _cp(lambda q: f[4][q], w50, p))
        f7n = cache(lambda p: f[7][p] + _cp(lambda q: f[6][q], w70, p))
        f8n = cache(lambda p: f[8][p] + _cp(lambda q: f[10][q], w80, p))
        f10n = cache(lambda p: f[10][p] + _cp(f8n, w100, p))
        for d in DFIX:
            out[b, 2, d] = f[2][d] + _cp(f5n, w20, d)
            out[b, 3, d] = f[3][d] + _cp(f7n, w30, d)
            out[b, 10, d] = f[10][d] + _cp(f8n, w100, d)
            out[b, 12, d] = f[12][d] + _cp(f8n, w120, d)
            out[b, 13, d] = f[13][d] + _cp(f10n, w130, d)


# ------------------------------------------------------- runner (cached) ---
_RUNNER = None


def _make_runner(nc):
    import jax
    from jax.sharding import Mesh, PartitionSpec, NamedSharding
    from jax.experimental.shard_map import shard_map

    install_neuronx_cc_hook()
    partition_name = nc.partition_id_tensor.name if nc.partition_id_tensor else None
    in_names, out_names, out_avals, zero_outs = [], [], [], []
    for alloc in nc.m.functions[0].allocations:
        if not isinstance(alloc, mybir.MemoryLocationSet):
            continue
        name = alloc.memorylocations[0].name
        if alloc.kind == "ExternalInput":
            if name != partition_name:
                in_names.append(name)
        elif alloc.kind == "ExternalOutput":
            out_names.append(name)
            shape = tuple(alloc.tensor_shape)
            dtype = mybir.dt.np(alloc.dtype)
            out_avals.append(jax.core.ShapedArray(shape, dtype))
            zero_outs.append(np.zeros(shape, dtype))
    n_params, n_outs = len(in_names), len(out_avals)
    all_in = list(in_names) + list(out_names)
    if partition_name is not None:
        all_in.append(partition_name)

    def _body(*args):
        operands = list(args)
        if partition_name is not None:
            operands.append(partition_id_tensor())
        return tuple(_bass_exec_p.bind(
            *operands, out_avals=tuple(out_avals), in_names=tuple(all_in),
            out_names=tuple(out_names),
            lowering_input_output_aliases=(),
            sim_require_finite=True, sim_require_nnan=True, nc=nc))

    devices = jax.devices()[:N_CORES]
    mesh = Mesh(np.asarray(devices), ("core",))
    sharded = jax.jit(
        shard_map(_body, mesh=mesh,
                  in_specs=(PartitionSpec("core"),) * (n_params + n_outs),
                  out_specs=(PartitionSpec("core"),) * n_outs,
                  check_rep=False),
        keep_unused=True)
    sharding = NamedSharding(mesh, PartitionSpec("core"))
    concat_zeros = [
        jax.device_put(np.zeros((N_CORES * z.shape[0], *z.shape[1:]), z.dtype),
                       sharding)
        for z in zero_outs]

    import jax as _jax

    def prepare(per_core_inputs):
        return [
            _jax.device_put(
                np.concatenate([np.asarray(m[n]) for m in per_core_inputs], axis=0),
                sharding)
            for n in in_names]

    def exec_dev(concat_in):
        return sharded(*concat_in, *concat_zeros)

    def run(per_core_inputs):
        outs = exec_dev(prepare(per_core_inputs))
        outs = [np.asarray(o) for o in outs]
        return [
            {n: outs[i].reshape(N_CORES, *out_avals[i].shape)[c]
             for i, n in enumerate(out_names)}
            for c in range(N_CORES)]

    run.prepare = prepare
    run.exec_dev = exec_dev
    return run


def get_runner():
    global _RUNNER
    if _RUNNER is None:
        nc = build_nc(reps=1)
        _RUNNER = _make_runner(nc)
    return _RUNNER


# ------------------------------------------------------------- entrypoint ---
def kernel(feature, weights):
    feature = np.ascontiguousarray(np.asarray(feature, dtype=np.float32))
    weights = np.ascontiguousarray(np.asarray(weights, dtype=np.float32))
    run = get_runner()
    bands = build_bands(weights)
    shards = make_shards(feature)
    in_maps = [{"xin": x, "bands": bands} for x in shards]
    results = run(in_maps)

    out = np.array(feature, copy=True)
    for c in range(N_CORES):
        b, q = divmod(c, 4)
        d0 = q * D_CHUNK
        d = results[c]["deltas"]
        for k, ch in enumerate(CH_OUT):
            # [h, w, d] -> [d, h, w]
            out[b, ch, d0:d0 + D_CHUNK] += \
                d[k].reshape(P, P, D_CHUNK).transpose(2, 0, 1).astype(np.float32)
    fix_boundaries(out, feature, weights)
    return out


if __name__ == "__main__":
    rng = np.random.default_rng(0)
    feature = rng.standard_normal((2, 17, 128, 128, 128), dtype=np.float32)
    weights = (rng.standard_normal((12, 3, 3, 3)) * 0.1).astype(np.float32)
    out = kernel(feature, weights)
    print("kernel ran, out shape", out.shape, out.dtype)
